# revision 3
# baseline (speedup 1.0000x reference)
"""Trainium2 Bass kernel for the attention-encoder (Bahdanau input attention
+ LSTM cell, T-step recurrence) -- series-expansion version.

Key restructuring vs the per-step baseline:

1. The LSTM state update does NOT depend on the attention output, so the
   T-step recurrence is decoupled from the expensive attention math.
   Phase A runs the bare LSTM chain (transposed layout: gates on
   partitions, states written directly into ring buffers -- no PE
   transposes), computing r1^T = We'^T [H;S] in bulk chunks of 8 steps
   and p = tanh(r1) into a resident [u, b, t] buffer.

2. The attention energies use the identity
       tanh(r1 + r2) = q + (1-q^2) * sum_{k>=1} (-1)^{k-1} q^{k-1} p^k
   with p = tanh(r1) (|p| <= 0.96 since LSTM states are bounded and We
   is small) and q = tanh(r2).  Truncating at K terms turns the
   [B,N,T,T] elementwise tanh + rank-1 contraction (the baseline's
   bottleneck) into K dense 128x128 matmuls per batch row:
       e[t,n] = sum_u ve*q[n,u] + sum_k sum_u p^k[t,u] * g_k[n,u],
       g_1 = ve (1-q^2),  g_{k+1} = g_k * (-q).
   Measured end-to-end (numpy, bf16 operands): K=6 -> rel_err 0.0016,
   K=4 -> 0.0032, versus the 2e-2 budget.

Sharding: pure data parallelism, batch 512 -> 64 rows per core.
"""

import numpy as np
import ml_dtypes
from contextlib import ExitStack

import concourse.bass as bass
import concourse.bacc as bacc
import concourse.tile as tile
from concourse import mybir
from concourse.bass_utils import run_bass_kernel_spmd

B, T, N, M = 512, 256, 128, 256
NCORES = 8
BL = B // NCORES  # 64 batch rows per core
M4 = 4 * M

BF16 = mybir.dt.bfloat16
F32 = mybir.dt.float32
TANH = mybir.ActivationFunctionType.Tanh
EXP = mybir.ActivationFunctionType.Exp
SQUARE = mybir.ActivationFunctionType.Square
ADD = mybir.AluOpType.add
MULT = mybir.AluOpType.mult
AXX = mybir.AxisListType.X

KSER = 4       # series terms (k = 1..KSER) + the k=0 (ve*q) term
RING = 32      # H/S state ring slots (state slot t = state entering step t)
CH = 8         # r1 bulk-chunk length (steps)
XCH = 16       # x^T DMA chunk length (steps)
BBLK = 4       # attention batch-rows per block (PSUM: 2 banks of e + dbuf)

# blob free-dim offsets (all [128, *] bf16, packed on host by _marshal)
OFF_UE = 0                        # Ue tiles   [p, 2(tt), 2(ut), 128]
OFF_WE = OFF_UE + 4 * 128         # We' tiles  [p, 4(j), 2(ut), 128]
OFF_WC = OFF_WE + 8 * 128         # Wc tiles   [p, 3(kt), 8(gt), 128]
OFF_ON = OFF_WC + 24 * 128        # ones       [p, 128]
BLOB_F = OFF_ON + 128


def build_nc(t_steps: int = T, with_bias: bool = False, repeats: int = 1,
             kser: int = KSER, debug_p: bool = False,
             parts: str = "all") -> bass.Bass:
    # parts: "all" | "A" (lstm+r1, no attention) | "lstm" (no r1 chunks either)
    #        | "B" (attention only, p left uninitialized)
    nc = bacc.Bacc(None)
    ts = t_steps
    n_tt = (ts + 127) // 128  # number of t-tiles in outputs

    x_p = nc.declare_dram_parameter("x_b", [BL, T, N], BF16, isOutput=False)
    xn_p = nc.declare_dram_parameter("x_n", [N, T, BL], BF16, isOutput=False)
    blob_p = nc.declare_dram_parameter("blob", [128, BLOB_F], BF16, isOutput=False)
    vesc_p = nc.declare_dram_parameter("vesc", [128, 4], F32, isOutput=False)
    hT_p = nc.declare_dram_parameter("hT0", [2, 128, BL], BF16, isOutput=False)
    sT_p = nc.declare_dram_parameter("sT0", [2, 128, BL], BF16, isOutput=False)
    if with_bias:
        bb_p = nc.declare_dram_parameter("biasT", [128, 8], F32, isOutput=False)
    out_p = nc.declare_dram_parameter("out", [BL, T, N], BF16, isOutput=True)
    if debug_p:
        dbg_p = nc.declare_dram_parameter("dbgp", [128, 2, BL, T], BF16, isOutput=True)
        dbg_h = nc.declare_dram_parameter("dbgh", [128, 2, RING, BL], BF16, isOutput=True)
        dbg_s = nc.declare_dram_parameter("dbgs", [128, 2, RING, BL], BF16, isOutput=True)

    with tile.TileContext(nc) as tc, ExitStack() as ctx:
        singles = ctx.enter_context(tc.tile_pool(name="singles", bufs=1))

        blob = singles.tile([128, BLOB_F], BF16)
        vesc = singles.tile([128, 4], F32)          # [ve0, ve1, -ve0, -ve1]
        pbuf = singles.tile([128, 2, BL, T], BF16)  # p[u(2 tiles), b, t]
        Hring = singles.tile([128, 2, RING, BL], BF16)
        Sring = singles.tile([128, 2, RING, BL], BF16)
        if with_bias:
            bb_s = singles.tile([128, 8], F32)
            nc.sync.dma_start(out=bb_s, in_=bb_p[:])

        ue_s = blob[:, OFF_UE:OFF_WE].rearrange("p (tt ut c) -> p tt ut c", tt=2, ut=2)
        we_s = blob[:, OFF_WE:OFF_WC].rearrange("p (j ut c) -> p j ut c", j=4, ut=2)
        wc_s = blob[:, OFF_WC:OFF_ON].rearrange("p (kt gt c) -> p kt gt c", kt=3, gt=8)
        ones_s = blob[:, OFF_ON:BLOB_F]

        nc.sync.dma_start(out=blob, in_=blob_p[:])
        nc.sync.dma_start(out=vesc, in_=vesc_p[:])

        # SBUF pools (whole-kernel scope)
        xtp = ctx.enter_context(tc.tile_pool(name="xtp", bufs=3))
        gatep = ctx.enter_context(tc.tile_pool(name="gatep", bufs=2))
        xnat = ctx.enter_context(tc.tile_pool(name="xnat", bufs=2 * BBLK))
        gpool = ctx.enter_context(tc.tile_pool(name="gpool", bufs=2))
        ppool = ctx.enter_context(tc.tile_pool(name="ppool", bufs=2))
        epool = ctx.enter_context(tc.tile_pool(name="epool", bufs=2))
        spool = ctx.enter_context(tc.tile_pool(name="spool", bufs=2))
        opool = ctx.enter_context(tc.tile_pool(name="opool", bufs=4))

        for rep in range(repeats):
            # initial (doubled) states into ring slot 0
            nc.sync.dma_start(out=Hring[:, :, 0, :], in_=hT_p.rearrange("m p b -> p m b"))
            nc.sync.dma_start(out=Sring[:, :, 0, :], in_=sT_p.rearrange("m p b -> p m b"))

            # ============ Phase A: LSTM chain + bulk r1 -> p ============
            if parts == "B":
                nc.vector.memset(pbuf[:, :, 0, 0:2], 0.0)
            else:
              with tc.tile_pool(name="zps", bufs=3, space="PSUM") as zps, \
                   tc.tile_pool(name="r1ps", bufs=3, space="PSUM") as r1ps:
                def r1_burst(c0, ut):
                    s0 = c0 % RING
                    r1p = r1ps.tile([128, CH, BL], F32, tag="r1")
                    for j in range(4):
                        ring = Hring if j < 2 else Sring
                        rhs = ring[:, j % 2, s0:s0 + CH, :]
                        nc.tensor.matmul(
                            r1p, lhsT=we_s[:, j, ut, :],
                            rhs=rhs.rearrange("p c b -> p (c b)"),
                            start=(j == 0), stop=(j == 3))
                    # p = tanh(r1) stored [u, b, t] (t innermost)
                    nc.scalar.activation(
                        pbuf[:, ut, :, c0:c0 + CH],
                        r1p.rearrange("p c b -> p b c"), TANH)

                xt_tile = None
                for t in range(ts):
                    if t % XCH == 0:
                        xt_tile = xtp.tile([128, XCH, BL], BF16, tag="xt")
                        nc.sync.dma_start(out=xt_tile, in_=xn_p[:, t:t + XCH, :])
                    sr = t % RING
                    sw = (t + 1) % RING

                    # NOTE: start=True clears has_written for the WHOLE PSUM
                    # bank, so the 8 gate regions in this bank must form ONE
                    # accumulation group: start only on the first matmul.
                    zp = zps.tile([128, 8, BL], F32, tag="z")
                    nox = parts == "lstm_nox"
                    if not nox:
                        # x-part (no h dependency; fills PE while chain waits)
                        for gt in range(8):
                            nc.tensor.matmul(zp[:, gt, :], lhsT=wc_s[:, 0, gt, :],
                                             rhs=xt_tile[:, t % XCH, :],
                                             start=(gt == 0), stop=False)
                    # h-part: f,i,g gate tiles (0..5) first, o tiles (6,7) last
                    for gt in (0, 1, 2, 3, 4, 5, 6, 7):
                        for kt in range(2):
                            nc.tensor.matmul(zp[:, gt, :], lhsT=wc_s[:, 1 + kt, gt, :],
                                             rhs=Hring[:, kt, sr, :],
                                             start=(nox and gt == 0 and kt == 0),
                                             stop=(gt == 7 and kt == 1))
                    if with_bias:
                        nc.vector.tensor_tensor(
                            zp, zp,
                            bb_s.rearrange("p g -> p g 1").to_broadcast([128, 8, BL]),
                            ADD)

                    # gates: tanh(0.5*z); g-gate columns pre-doubled on host
                    if parts == "lstm_merged":
                        t_all = gatep.tile([128, 8, BL], BF16, tag="tfig")
                        nc.scalar.activation(t_all, zp, TANH, scale=0.5)
                        t_fig = t_all[:, 0:6, :]
                        t_o = t_all[:, 6:8, :]
                    else:
                        t_fig = gatep.tile([128, 6, BL], BF16, tag="tfig")
                        nc.scalar.activation(t_fig, zp[:, 0:6, :], TANH, scale=0.5)
                        t_o = gatep.tile([128, 2, BL], BF16, tag="to")
                        nc.scalar.activation(t_o, zp[:, 6:8, :], TANH, scale=0.5)

                    # S' = 0.5*(t_f+1)*S + (t_i+1)*t_g ; H' = (t_o+1)*tanh(0.5 S')
                    v = gatep.tile([128, 2, BL], BF16, tag="v")
                    nc.vector.scalar_tensor_tensor(v, t_fig[:, 0:2, :], 1.0,
                                                   Sring[:, :, sr, :], ADD, MULT)
                    qq = gatep.tile([128, 2, BL], BF16, tag="qq")
                    nc.vector.scalar_tensor_tensor(qq, t_fig[:, 2:4, :], 1.0,
                                                   t_fig[:, 4:6, :], ADD, MULT)
                    nc.vector.scalar_tensor_tensor(Sring[:, :, sw, :], v, 0.5,
                                                   qq, MULT, ADD)
                    if parts == "lstm_nots":
                        # timing probe only: wrong math, tanh_s hop removed
                        nc.vector.scalar_tensor_tensor(Hring[:, :, sw, :], t_o, 1.0,
                                                       Sring[:, :, sw, :], ADD, MULT)
                    else:
                        tanh_s = gatep.tile([128, 2, BL], BF16, tag="tanhs")
                        nc.scalar.activation(tanh_s, Sring[:, :, sw, :], TANH, scale=0.5)
                        nc.vector.scalar_tensor_tensor(Hring[:, :, sw, :], t_o, 1.0,
                                                       tanh_s, ADD, MULT)

                    # bulk r1 chunk once the last step of the chunk is queued:
                    # r1[u, tc, b] for tc in [c*CH, c*CH+CH) uses state slots
                    # (c*CH + i) % RING which are contiguous (CH | RING).
                    # The two u-tiles are emitted on consecutive steps so each
                    # 4-matmul burst fits in the PE idle gap of one step
                    # instead of head-of-line-blocking the next chain step.
                    if not parts.startswith("lstm") and t >= CH - 1:
                        ph = (t - (CH - 1)) % CH
                        if ph in (0, 1) and t - ph + 1 >= CH:
                            r1_burst(t - (CH - 1) - ph, ph)
                if not parts.startswith("lstm") and ts >= CH:
                    r1_burst(ts - CH, 1)  # trailing u-tile-1 burst

            if debug_p:
                nc.sync.dma_start(out=dbg_p[:], in_=pbuf)
                nc.sync.dma_start(out=dbg_h[:], in_=Hring)
                nc.sync.dma_start(out=dbg_s[:], in_=Sring)

            # ============ Phase B: series attention + softmax ============
            if parts == "A" or parts.startswith("lstm"):
                ot0 = opool.tile([128, 2, N], BF16, tag="ot")
                nc.vector.tensor_copy(ot0[:, 0, 0:BL], Hring[:, 0, 0, :])
                nc.sync.dma_start(out=out_p[0, 0:128, :], in_=ot0[:, 0, :])
                continue
            with tc.tile_pool(name="r2ps", bufs=2, space="PSUM") as r2ps, \
                 tc.tile_pool(name="eps", bufs=2, space="PSUM") as eps:
                for blk in range(BL // BBLK):
                    b0 = blk * BBLK
                    xb_tiles = []
                    for bi in range(BBLK):
                        xt = xnat.tile([128, 2, N], BF16, tag="xnat")
                        nc.sync.dma_start(
                            out=xt,
                            in_=x_p[b0 + bi].rearrange("(t p) n -> p t n", p=128))
                        xb_tiles.append(xt)

                    # r2^T[u, n] per row -> nq = tanh(-r2)
                    nqb = gpool.tile([128, 2, BBLK, N], BF16, tag="nq")
                    for ut in range(2):
                        r2p = r2ps.tile([128, BBLK, N], F32, tag="r2")
                        for bi in range(BBLK):
                            for tt in range(2):
                                nc.tensor.matmul(r2p[:, bi, :],
                                                 lhsT=ue_s[:, tt, ut, :],
                                                 rhs=xb_tiles[bi][:, tt, :],
                                                 start=(tt == 0), stop=(tt == 1))
                        nc.scalar.activation(nqb[:, ut], r2p, TANH, scale=-1.0)

                    # g-chain: qve = ve*q ; g1 = ve(1-q^2) ; g_{k+1} = g_k * (-q)
                    qve = gpool.tile([128, 2, BBLK, N], BF16, tag="qve")
                    for ut in range(2):
                        nc.vector.tensor_scalar(
                            out=qve[:, ut], in0=nqb[:, ut],
                            scalar1=vesc[:, 2 + ut:3 + ut], scalar2=None, op0=MULT)
                    nsq = gpool.tile([128, 2, BBLK, N], BF16, tag="nsq")
                    nc.vector.tensor_tensor(nsq, nqb, nqb, MULT)
                    g1 = gpool.tile([128, 2, BBLK, N], BF16, tag="g1")
                    for ut in range(2):
                        nc.vector.tensor_scalar(
                            out=g1[:, ut], in0=nsq[:, ut],
                            scalar1=vesc[:, 2 + ut:3 + ut],
                            scalar2=vesc[:, ut:ut + 1], op0=MULT, op1=ADD)
                    gs = {0: qve, 1: g1}
                    for k in range(2, kser + 1):
                        gk = gpool.tile([128, 2, BBLK, N], BF16, tag=f"g{k}")
                        nc.vector.tensor_tensor(gk, gs[k - 1], nqb, MULT)
                        gs[k] = gk

                    ep = eps.tile([128, BBLK, 2, N], F32, tag="e")
                    for bi in range(BBLK):
                        b = b0 + bi
                        # powers of p for this row: even on ACT, odd on DVE
                        pws = {1: pbuf[:, :, b, :]}
                        for k in range(2, kser + 1):
                            pk = ppool.tile([128, 2, T], BF16, tag=f"pw{k}")
                            if k == 2:
                                nc.scalar.activation(pk[:, :, :ts],
                                                     pws[1][:, :, :ts], SQUARE)
                            else:
                                # k>2 via DVE products; phase B is ACT-bound
                                nc.vector.tensor_tensor(pk[:, :, :ts],
                                                        pws[k - 2][:, :, :ts],
                                                        pws[2][:, :, :ts], MULT)
                            pws[k] = pk
                        for tt in range(n_tt):
                            tc0 = tt * 128
                            tcols = min(128, ts - tc0)
                            for k in range(kser + 1):
                                for ut in range(2):
                                    if k == 0:
                                        lhsT = ones_s[:, :tcols]
                                    else:
                                        lhsT = pws[k][:, ut, tc0:tc0 + tcols]
                                    nc.tensor.matmul(
                                        ep[:tcols, bi, tt, :], lhsT=lhsT,
                                        rhs=gs[k][:, ut, bi, :],
                                        start=(k == 0 and ut == 0),
                                        stop=(k == kser and ut == 1))

                    # softmax over n (free axis) + output alpha * x
                    expb = epool.tile([128, BBLK, 2, N], BF16, tag="exp")
                    red = spool.tile([128, BBLK, 2, 1], F32, tag="red")
                    rec = spool.tile([128, BBLK, 2, 1], F32, tag="rec")
                    for tt in range(n_tt):
                        tcols = min(128, ts - tt * 128)
                        nc.scalar.activation(expb[:tcols, :, tt, :],
                                             ep[:tcols, :, tt, :], EXP)
                        nc.vector.reduce_sum(red[:tcols, :, tt, :],
                                             expb[:tcols, :, tt, :], axis=AXX)
                    nc.vector.reciprocal(rec.rearrange("p b t o -> p (b t) o"),
                                         red.rearrange("p b t o -> p (b t) o"))
                    for bi in range(BBLK):
                        b = b0 + bi
                        ot = opool.tile([128, 2, N], BF16, tag="ot")
                        for tt in range(n_tt):
                            tc0 = tt * 128
                            tcols = min(128, ts - tc0)
                            nc.vector.scalar_tensor_tensor(
                                ot[:tcols, tt, :], expb[:tcols, bi, tt, :],
                                rec[:tcols, bi, tt, :], xb_tiles[bi][:tcols, tt, :],
                                MULT, MULT)
                            nc.sync.dma_start(out=out_p[b, tc0:tc0 + tcols, :],
                                              in_=ot[:tcols, tt, :])

    nc.compile()
    return nc


def _marshal(x, s, h, We, Ue, ve, Wk, Wr, b):
    """Host-side input prep (sharding + weight prepacking, no x-dependent math)."""
    bf = ml_dtypes.bfloat16
    x_bf = x.astype(bf)                                   # [B, T, N]
    h2 = (h.astype(np.float32) * 2.0)   # doubled states
    s2 = (s.astype(np.float32) * 2.0)
    hT = np.ascontiguousarray(h2.astype(bf).T)            # [M, B]
    sT = np.ascontiguousarray(s2.astype(bf).T)

    # Ue tiles [tt, 128, ut, 128] -> blob [128, 2, 2, 128]
    ue_w = Ue.astype(np.float32).reshape(2, 128, 2, 128).transpose(1, 0, 2, 3)
    # We' = 0.5*We (doubled states), tiles [j, 128, ut, 128] -> [128, 4, 2, 128]
    we_w = (We.astype(np.float32) * 0.5).reshape(4, 128, 2, 128).transpose(1, 0, 2, 3)
    # Wc: concat(Wk, 0.5*Wr) rows; columns regrouped (f0 f1 i0 i1 g0 g1 o0 o1),
    # g columns doubled so one tanh(0.5 z) serves all gates.
    wc = np.concatenate([Wk, Wr * 0.5], axis=0).astype(np.float32)  # [384, 1024]
    gt_cols = [(256, 384), (384, 512), (0, 128), (128, 256),
               (512, 640), (640, 768), (768, 896), (896, 1024)]
    wc_w = np.empty((128, 3, 8, 128), np.float32)
    for kt in range(3):
        for gt, (c0, c1) in enumerate(gt_cols):
            blkw = wc[kt * 128:(kt + 1) * 128, c0:c1]
            if gt in (4, 5):
                blkw = blkw * 2.0
            wc_w[:, kt, gt, :] = blkw
    ones = np.ones((128, 128), np.float32)

    blob = np.concatenate([
        ue_w.reshape(128, -1), we_w.reshape(128, -1),
        wc_w.reshape(128, -1), ones,
    ], axis=1).astype(bf)

    vef = ve[:, 0].astype(np.float32)
    vesc = np.stack([vef[:128], vef[128:], -vef[:128], -vef[128:]], axis=1)
    vesc = np.ascontiguousarray(vesc, dtype=np.float32)

    with_bias = bool(np.any(b))
    bias2 = np.concatenate([b[256:512], b[0:256], b[512:768] * 2.0, b[768:1024]])
    biasT = np.ascontiguousarray(bias2.reshape(8, 128).T, dtype=np.float32)

    in_maps = []
    for i in range(NCORES):
        sl = slice(i * BL, (i + 1) * BL)
        m = {
            "x_b": np.ascontiguousarray(x_bf[sl]),
            "x_n": np.ascontiguousarray(x_bf[sl].transpose(2, 1, 0)),
            "blob": blob,
            "vesc": vesc,
            "hT0": np.ascontiguousarray(hT[:, sl].reshape(2, 128, BL)),
            "sT0": np.ascontiguousarray(sT[:, sl].reshape(2, 128, BL)),
        }
        if with_bias:
            m["biasT"] = biasT
        in_maps.append(m)
    return in_maps, with_bias


def kernel(**inputs) -> np.ndarray:
    x = np.asarray(inputs["x"])
    s = np.asarray(inputs["s"])
    h = np.asarray(inputs["h"])
    We = np.asarray(inputs["We"])
    Ue = np.asarray(inputs["Ue"])
    ve = np.asarray(inputs["ve"])
    Wk = np.asarray(inputs["Wk"])
    Wr = np.asarray(inputs["Wr"])
    b = np.asarray(inputs["b"])

    in_maps, with_bias = _marshal(x, s, h, We, Ue, ve, Wk, Wr, b)
    nc = build_nc(T, with_bias=with_bias)
    res = run_bass_kernel_spmd(nc, in_maps, core_ids=list(range(NCORES)))
    out = np.concatenate([r["out"] for r in res.results], axis=0)
    return out.astype(np.float32)


if __name__ == "__main__":
    rng = np.random.default_rng(0)
    demo = {
        "x": rng.standard_normal((B, T, N), dtype=np.float32),
        "s": rng.standard_normal((B, M), dtype=np.float32) * 0.1,
        "h": rng.standard_normal((B, M), dtype=np.float32) * 0.1,
        "We": rng.standard_normal((2 * M, T), dtype=np.float32) / np.sqrt(2 * M),
        "Ue": rng.standard_normal((T, T), dtype=np.float32) / np.sqrt(T),
        "ve": rng.standard_normal((T, 1), dtype=np.float32) / np.sqrt(T),
        "Wk": rng.standard_normal((N, M4), dtype=np.float32) / np.sqrt(N),
        "Wr": rng.standard_normal((M, M4), dtype=np.float32) / np.sqrt(M),
        "b": np.zeros((M4,), dtype=np.float32),
    }
    out = kernel(**demo)
    print(out.shape, out.dtype)


# revision 4
# speedup vs baseline: 1.0807x; 1.0807x over previous
"""Trainium2 Bass kernel for the attention-encoder (Bahdanau input attention
+ LSTM cell, T-step recurrence) -- series-expansion version.

Key restructuring vs the per-step baseline:

1. The LSTM state update does NOT depend on the attention output, so the
   T-step recurrence is decoupled from the expensive attention math.
   Phase A runs the bare LSTM chain (transposed layout: gates on
   partitions, states written directly into ring buffers -- no PE
   transposes), computing r1^T = We'^T [H;S] in bulk chunks of 8 steps
   and p = tanh(r1) into a resident [u, b, t] buffer.

2. The attention energies use the identity
       tanh(r1 + r2) = q + (1-q^2) * sum_{k>=1} (-1)^{k-1} q^{k-1} p^k
   with p = tanh(r1) (|p| <= 0.96 since LSTM states are bounded and We
   is small) and q = tanh(r2).  Truncating at K terms turns the
   [B,N,T,T] elementwise tanh + rank-1 contraction (the baseline's
   bottleneck) into K dense 128x128 matmuls per batch row:
       e[t,n] = sum_u ve*q[n,u] + sum_k sum_u p^k[t,u] * g_k[n,u],
       g_1 = ve (1-q^2),  g_{k+1} = g_k * (-q).
   Measured end-to-end (numpy, bf16 operands): K=6 -> rel_err 0.0016,
   K=4 -> 0.0032, versus the 2e-2 budget.

Sharding: pure data parallelism, batch 512 -> 64 rows per core.
"""

import numpy as np
import ml_dtypes
from contextlib import ExitStack

import concourse.bass as bass
import concourse.bacc as bacc
import concourse.tile as tile
from concourse import mybir
from concourse.bass_utils import run_bass_kernel_spmd

B, T, N, M = 512, 256, 128, 256
NCORES = 8
BL = B // NCORES  # 64 batch rows per core
M4 = 4 * M

BF16 = mybir.dt.bfloat16
F32 = mybir.dt.float32
TANH = mybir.ActivationFunctionType.Tanh
EXP = mybir.ActivationFunctionType.Exp
SQUARE = mybir.ActivationFunctionType.Square
ADD = mybir.AluOpType.add
MULT = mybir.AluOpType.mult
AXX = mybir.AxisListType.X

KSER = 4       # series terms (k = 1..KSER) + the k=0 (ve*q) term
RING = 32      # H/S state ring slots (state slot t = state entering step t)
CH = 8         # r1 bulk-chunk length (steps)
XCH = 16       # x^T DMA chunk length (steps)
BBLK = 4       # attention batch-rows per block (PSUM: 2 banks of e + dbuf)

# blob free-dim offsets (all [128, *] bf16, packed on host by _marshal)
OFF_UE = 0                        # Ue tiles   [p, 2(tt), 2(ut), 128]
OFF_WE = OFF_UE + 4 * 128         # We' tiles  [p, 4(j), 2(ut), 128]
OFF_WC = OFF_WE + 8 * 128         # Wc tiles   [p, 3(kt), 8(gt), 128]
OFF_ON = OFF_WC + 24 * 128        # ones       [p, 128]
BLOB_F = OFF_ON + 128


def build_nc(t_steps: int = T, with_bias: bool = False, repeats: int = 1,
             kser: int = KSER, debug_p: bool = False,
             parts: str = "all") -> bass.Bass:
    # parts: "all" | "A" (lstm+r1, no attention) | "lstm" (no r1 chunks either)
    #        | "B" (attention only, p left uninitialized)
    nc = bacc.Bacc(None)
    ts = t_steps
    n_tt = (ts + 127) // 128  # number of t-tiles in outputs

    x_p = nc.declare_dram_parameter("x_b", [BL, T, N], BF16, isOutput=False)
    xn_p = nc.declare_dram_parameter("x_n", [N, T, BL], BF16, isOutput=False)
    blob_p = nc.declare_dram_parameter("blob", [128, BLOB_F], BF16, isOutput=False)
    vesc_p = nc.declare_dram_parameter("vesc", [128, 4], F32, isOutput=False)
    hT_p = nc.declare_dram_parameter("hT0", [2, 128, BL], BF16, isOutput=False)
    sT_p = nc.declare_dram_parameter("sT0", [2, 128, BL], BF16, isOutput=False)
    if with_bias:
        bb_p = nc.declare_dram_parameter("biasT", [128, 8], F32, isOutput=False)
    out_p = nc.declare_dram_parameter("out", [BL, T, N], BF16, isOutput=True)
    if debug_p:
        dbg_p = nc.declare_dram_parameter("dbgp", [128, 2, BL, T], BF16, isOutput=True)
        dbg_h = nc.declare_dram_parameter("dbgh", [128, 2, RING, BL], BF16, isOutput=True)
        dbg_s = nc.declare_dram_parameter("dbgs", [128, 2, RING, BL], BF16, isOutput=True)

    with tile.TileContext(nc) as tc, ExitStack() as ctx:
        singles = ctx.enter_context(tc.tile_pool(name="singles", bufs=1))

        blob = singles.tile([128, BLOB_F], BF16)
        vesc = singles.tile([128, 4], F32)          # [ve0, ve1, -ve0, -ve1]
        pbuf = singles.tile([128, 2, BL, T], BF16)  # p[u(2 tiles), b, t]
        Hring = singles.tile([128, 2, RING, BL], BF16)
        Sring = singles.tile([128, 2, RING, BL], BF16)
        if with_bias:
            bb_s = singles.tile([128, 8], F32)
            nc.sync.dma_start(out=bb_s, in_=bb_p[:])

        ue_s = blob[:, OFF_UE:OFF_WE].rearrange("p (tt ut c) -> p tt ut c", tt=2, ut=2)
        we_s = blob[:, OFF_WE:OFF_WC].rearrange("p (j ut c) -> p j ut c", j=4, ut=2)
        wc_s = blob[:, OFF_WC:OFF_ON].rearrange("p (kt gt c) -> p kt gt c", kt=3, gt=8)
        ones_s = blob[:, OFF_ON:BLOB_F]

        nc.sync.dma_start(out=blob, in_=blob_p[:])
        nc.sync.dma_start(out=vesc, in_=vesc_p[:])

        # SBUF pools (whole-kernel scope)
        xtp = ctx.enter_context(tc.tile_pool(name="xtp", bufs=3))
        gatep = ctx.enter_context(tc.tile_pool(name="gatep", bufs=2))
        xnat = ctx.enter_context(tc.tile_pool(name="xnat", bufs=3 * BBLK))
        gpool = ctx.enter_context(tc.tile_pool(name="gpool", bufs=3))
        ppool = ctx.enter_context(tc.tile_pool(name="ppool", bufs=3))
        epool = ctx.enter_context(tc.tile_pool(name="epool", bufs=3))
        spool = ctx.enter_context(tc.tile_pool(name="spool", bufs=3))
        opool = ctx.enter_context(tc.tile_pool(name="opool", bufs=4))

        for rep in range(repeats):
            # initial (doubled) states into ring slot 0
            nc.sync.dma_start(out=Hring[:, :, 0, :], in_=hT_p.rearrange("m p b -> p m b"))
            nc.sync.dma_start(out=Sring[:, :, 0, :], in_=sT_p.rearrange("m p b -> p m b"))

            # ============ Phase A: LSTM chain + bulk r1 -> p ============
            with tc.tile_pool(name="zps", bufs=2, space="PSUM") as zps, \
                 tc.tile_pool(name="r1ps", bufs=2, space="PSUM") as r1ps, \
                 tc.tile_pool(name="r2ps", bufs=2, space="PSUM") as r2ps, \
                 tc.tile_pool(name="eps", bufs=2, space="PSUM") as eps:

              # ---- attention emission machinery (per-(t-tile, block) thunks).
              # Each (blk, tt) is a list of 7 thunks sized to fit one LSTM
              # step's engine slack, so pass tt=0 drip-feeds into phase A's
              # idle time once p's first t-tile is complete (warms the PE).
              def attn_sched(tt_idx, blk):
                  b0 = blk * BBLK
                  tc0 = tt_idx * 128
                  tcols = min(128, ts - tc0)
                  state = {}

                  def th_r2():
                      xb = []
                      for bi in range(BBLK):
                          xt = xnat.tile([128, 2, N], BF16, tag="xnat")
                          nc.sync.dma_start(
                              out=xt,
                              in_=x_p[b0 + bi].rearrange("(t p) n -> p t n", p=128))
                          xb.append(xt)
                      state["xb"] = xb
                      nqb = gpool.tile([128, 2, BBLK, N], BF16, tag="nq")
                      for ut in range(2):
                          r2p = r2ps.tile([128, BBLK, N], F32, tag="r2")
                          for bi in range(BBLK):
                              for tk in range(2):
                                  nc.tensor.matmul(r2p[:, bi, :],
                                                   lhsT=ue_s[:, tk, ut, :],
                                                   rhs=xb[bi][:, tk, :],
                                                   start=(tk == 0), stop=(tk == 1))
                          nc.scalar.activation(nqb[:, ut], r2p, TANH, scale=-1.0)
                      state["nq"] = nqb

                  def th_g1():
                      nqb = state["nq"]
                      qve = gpool.tile([128, 2, BBLK, N], BF16, tag="qve")
                      for ut in range(2):
                          nc.vector.tensor_scalar(
                              out=qve[:, ut], in0=nqb[:, ut],
                              scalar1=vesc[:, 2 + ut:3 + ut], scalar2=None, op0=MULT)
                      nsq = gpool.tile([128, 2, BBLK, N], BF16, tag="nsq")
                      nc.vector.tensor_tensor(nsq, nqb, nqb, MULT)
                      g1 = gpool.tile([128, 2, BBLK, N], BF16, tag="g1")
                      for ut in range(2):
                          nc.vector.tensor_scalar(
                              out=g1[:, ut], in0=nsq[:, ut],
                              scalar1=vesc[:, 2 + ut:3 + ut],
                              scalar2=vesc[:, ut:ut + 1], op0=MULT, op1=ADD)
                      state["gs"] = {0: qve, 1: g1}

                  def th_g2():
                      nqb = state["nq"]
                      gs = state["gs"]
                      for k in range(2, kser + 1):
                          gk = gpool.tile([128, 2, BBLK, N], BF16, tag=f"g{k}")
                          nc.vector.tensor_tensor(gk, gs[k - 1], nqb, MULT)
                          gs[k] = gk
                      ep_t = eps.tile([128, BBLK, N], F32, tag="e")
                      state["ep"] = ep_t

                  def mk_b(bi):
                      def th_b():
                          b = b0 + bi
                          p1h = pbuf[:, :, b, tc0:tc0 + tcols]
                          pws = {1: p1h}
                          for k in range(2, kser + 1):
                              pk = ppool.tile([128, 2, 128], BF16, tag=f"pw{k}")
                              if k == 2:
                                  nc.scalar.activation(pk[:, :, :tcols], p1h, SQUARE)
                              else:
                                  nc.vector.tensor_tensor(pk[:, :, :tcols],
                                                          pws[k - 2],
                                                          pws[2], MULT)
                              pws[k] = pk[:, :, :tcols]
                          ep = state["ep"]
                          for k in range(kser + 1):
                              for ut in range(2):
                                  if k == 0:
                                      lhsT = ones_s[:, :tcols]
                                  else:
                                      lhsT = pws[k][:, ut, :]
                                  nc.tensor.matmul(
                                      ep[:tcols, bi, :], lhsT=lhsT,
                                      rhs=state["gs"][k][:, ut, bi, :],
                                      start=(k == 0 and ut == 0),
                                      stop=(k == kser and ut == 1))
                      return th_b

                  def th_sm():
                      ep = state["ep"]
                      expb = epool.tile([128, BBLK, N], BF16, tag="exp")
                      nc.scalar.activation(expb[:tcols], ep[:tcols], EXP)
                      red = spool.tile([128, BBLK, 1], F32, tag="red")
                      nc.vector.reduce_sum(red[:tcols], expb[:tcols], axis=AXX)
                      rec = spool.tile([128, BBLK, 1], F32, tag="rec")
                      nc.vector.reciprocal(rec, red)
                      for bi in range(BBLK):
                          b = b0 + bi
                          ot = opool.tile([128, N], BF16, tag="ot")
                          nc.vector.scalar_tensor_tensor(
                              ot[:tcols], expb[:tcols, bi, :],
                              rec[:tcols, bi, :],
                              state["xb"][bi][:tcols, tt_idx, :], MULT, MULT)
                          nc.sync.dma_start(out=out_p[b, tc0:tc0 + tcols, :],
                                            in_=ot[:tcols])

                  return [th_r2, th_g1, th_g2, mk_b(0), mk_b(1), mk_b(2), mk_b(3), th_sm]

              do_attn = parts in ("all", "B")
              interleave = do_attn and ts == T
              pending = []
              if interleave:
                  for blk in range(BL // BBLK):
                      pending.extend(attn_sched(0, blk))
              pending.reverse()

              if parts == "B":
                nc.vector.memset(pbuf[:, :, 0, 0:2], 0.0)
              else:
                def r1_burst(c0, ut):
                    s0 = c0 % RING
                    r1p = r1ps.tile([128, CH, BL], F32, tag="r1")
                    for j in range(4):
                        ring = Hring if j < 2 else Sring
                        rhs = ring[:, j % 2, s0:s0 + CH, :]
                        nc.tensor.matmul(
                            r1p, lhsT=we_s[:, j, ut, :],
                            rhs=rhs.rearrange("p c b -> p (c b)"),
                            start=(j == 0), stop=(j == 3))
                    # p = tanh(r1) stored [u, b, t] (t innermost)
                    nc.scalar.activation(
                        pbuf[:, ut, :, c0:c0 + CH],
                        r1p.rearrange("p c b -> p b c"), TANH)

                xt_tile = None
                for t in range(ts):
                    if t % XCH == 0:
                        xt_tile = xtp.tile([128, XCH, BL], BF16, tag="xt")
                        nc.sync.dma_start(out=xt_tile, in_=xn_p[:, t:t + XCH, :])
                    sr = t % RING
                    sw = (t + 1) % RING

                    # NOTE: start=True clears has_written for the WHOLE PSUM
                    # bank, so the 8 gate regions in this bank must form ONE
                    # accumulation group: start only on the first matmul.
                    zp = zps.tile([128, 8, BL], F32, tag="z")
                    nox = parts == "lstm_nox"
                    if not nox:
                        # x-part (no h dependency; fills PE while chain waits)
                        for gt in range(8):
                            nc.tensor.matmul(zp[:, gt, :], lhsT=wc_s[:, 0, gt, :],
                                             rhs=xt_tile[:, t % XCH, :],
                                             start=(gt == 0), stop=False)
                    # h-part: f,i,g gate tiles (0..5) first, o tiles (6,7) last
                    for gt in (0, 1, 2, 3, 4, 5, 6, 7):
                        for kt in range(2):
                            nc.tensor.matmul(zp[:, gt, :], lhsT=wc_s[:, 1 + kt, gt, :],
                                             rhs=Hring[:, kt, sr, :],
                                             start=(nox and gt == 0 and kt == 0),
                                             stop=(gt == 7 and kt == 1))
                    if with_bias:
                        nc.vector.tensor_tensor(
                            zp, zp,
                            bb_s.rearrange("p g -> p g 1").to_broadcast([128, 8, BL]),
                            ADD)

                    # gates: tanh(0.5*z); g-gate columns pre-doubled on host
                    if parts == "lstm_merged":
                        t_all = gatep.tile([128, 8, BL], BF16, tag="tfig")
                        nc.scalar.activation(t_all, zp, TANH, scale=0.5)
                        t_fig = t_all[:, 0:6, :]
                        t_o = t_all[:, 6:8, :]
                    else:
                        t_fig = gatep.tile([128, 6, BL], BF16, tag="tfig")
                        nc.scalar.activation(t_fig, zp[:, 0:6, :], TANH, scale=0.5)
                        t_o = gatep.tile([128, 2, BL], BF16, tag="to")
                        nc.scalar.activation(t_o, zp[:, 6:8, :], TANH, scale=0.5)

                    # S' = 0.5*(t_f+1)*S + (t_i+1)*t_g ; H' = (t_o+1)*tanh(0.5 S')
                    v = gatep.tile([128, 2, BL], BF16, tag="v")
                    nc.vector.scalar_tensor_tensor(v, t_fig[:, 0:2, :], 1.0,
                                                   Sring[:, :, sr, :], ADD, MULT)
                    qq = gatep.tile([128, 2, BL], BF16, tag="qq")
                    nc.vector.scalar_tensor_tensor(qq, t_fig[:, 2:4, :], 1.0,
                                                   t_fig[:, 4:6, :], ADD, MULT)
                    nc.vector.scalar_tensor_tensor(Sring[:, :, sw, :], v, 0.5,
                                                   qq, MULT, ADD)
                    if parts == "lstm_nots":
                        # timing probe only: wrong math, tanh_s hop removed
                        nc.vector.scalar_tensor_tensor(Hring[:, :, sw, :], t_o, 1.0,
                                                       Sring[:, :, sw, :], ADD, MULT)
                    else:
                        tanh_s = gatep.tile([128, 2, BL], BF16, tag="tanhs")
                        nc.scalar.activation(tanh_s, Sring[:, :, sw, :], TANH, scale=0.5)
                        nc.vector.scalar_tensor_tensor(Hring[:, :, sw, :], t_o, 1.0,
                                                       tanh_s, ADD, MULT)

                    # bulk r1 chunk once the last step of the chunk is queued:
                    # r1[u, tc, b] for tc in [c*CH, c*CH+CH) uses state slots
                    # (c*CH + i) % RING which are contiguous (CH | RING).
                    # The two u-tiles are emitted on consecutive steps so each
                    # 4-matmul burst fits in the PE idle gap of one step
                    # instead of head-of-line-blocking the next chain step.
                    if not parts.startswith("lstm") and t >= CH + 1:
                        ph = (t - (CH - 1)) % CH
                        if ph in (2, 3) and t - (CH - 1) - ph + CH <= ts - CH:
                            r1_burst(t - (CH - 1) - ph, ph - 2)
                    if pending and t >= 139:
                        pending.pop()()
                if not parts.startswith("lstm") and ts >= CH:
                    r1_burst(ts - CH, 0)  # final chunk, both u-tiles
                    r1_burst(ts - CH, 1)
                while pending:
                    pending.pop()()

              # ============ Phase B tail: remaining t-tile passes ==========
              if do_attn:
                  for tti in range(1 if interleave else 0, n_tt):
                      for blk in range(BL // BBLK):
                          for th in attn_sched(tti, blk):
                              th()

            if debug_p:
                nc.sync.dma_start(out=dbg_p[:], in_=pbuf)
                nc.sync.dma_start(out=dbg_h[:], in_=Hring)
                nc.sync.dma_start(out=dbg_s[:], in_=Sring)

    nc.compile()
    return nc


def _marshal(x, s, h, We, Ue, ve, Wk, Wr, b):
    """Host-side input prep (sharding + weight prepacking, no x-dependent math)."""
    bf = ml_dtypes.bfloat16
    x_bf = x.astype(bf)                                   # [B, T, N]
    h2 = (h.astype(np.float32) * 2.0)   # doubled states
    s2 = (s.astype(np.float32) * 2.0)
    hT = np.ascontiguousarray(h2.astype(bf).T)            # [M, B]
    sT = np.ascontiguousarray(s2.astype(bf).T)

    # Ue tiles [tt, 128, ut, 128] -> blob [128, 2, 2, 128]
    ue_w = Ue.astype(np.float32).reshape(2, 128, 2, 128).transpose(1, 0, 2, 3)
    # We' = 0.5*We (doubled states), tiles [j, 128, ut, 128] -> [128, 4, 2, 128]
    we_w = (We.astype(np.float32) * 0.5).reshape(4, 128, 2, 128).transpose(1, 0, 2, 3)
    # Wc: concat(Wk, 0.5*Wr) rows; columns regrouped (f0 f1 i0 i1 g0 g1 o0 o1),
    # g columns doubled so one tanh(0.5 z) serves all gates.
    wc = np.concatenate([Wk, Wr * 0.5], axis=0).astype(np.float32)  # [384, 1024]
    gt_cols = [(256, 384), (384, 512), (0, 128), (128, 256),
               (512, 640), (640, 768), (768, 896), (896, 1024)]
    wc_w = np.empty((128, 3, 8, 128), np.float32)
    for kt in range(3):
        for gt, (c0, c1) in enumerate(gt_cols):
            blkw = wc[kt * 128:(kt + 1) * 128, c0:c1]
            if gt in (4, 5):
                blkw = blkw * 2.0
            wc_w[:, kt, gt, :] = blkw
    ones = np.ones((128, 128), np.float32)

    blob = np.concatenate([
        ue_w.reshape(128, -1), we_w.reshape(128, -1),
        wc_w.reshape(128, -1), ones,
    ], axis=1).astype(bf)

    vef = ve[:, 0].astype(np.float32)
    vesc = np.stack([vef[:128], vef[128:], -vef[:128], -vef[128:]], axis=1)
    vesc = np.ascontiguousarray(vesc, dtype=np.float32)

    with_bias = bool(np.any(b))
    bias2 = np.concatenate([b[256:512], b[0:256], b[512:768] * 2.0, b[768:1024]])
    biasT = np.ascontiguousarray(bias2.reshape(8, 128).T, dtype=np.float32)

    in_maps = []
    for i in range(NCORES):
        sl = slice(i * BL, (i + 1) * BL)
        m = {
            "x_b": np.ascontiguousarray(x_bf[sl]),
            "x_n": np.ascontiguousarray(x_bf[sl].transpose(2, 1, 0)),
            "blob": blob,
            "vesc": vesc,
            "hT0": np.ascontiguousarray(hT[:, sl].reshape(2, 128, BL)),
            "sT0": np.ascontiguousarray(sT[:, sl].reshape(2, 128, BL)),
        }
        if with_bias:
            m["biasT"] = biasT
        in_maps.append(m)
    return in_maps, with_bias


def kernel(**inputs) -> np.ndarray:
    x = np.asarray(inputs["x"])
    s = np.asarray(inputs["s"])
    h = np.asarray(inputs["h"])
    We = np.asarray(inputs["We"])
    Ue = np.asarray(inputs["Ue"])
    ve = np.asarray(inputs["ve"])
    Wk = np.asarray(inputs["Wk"])
    Wr = np.asarray(inputs["Wr"])
    b = np.asarray(inputs["b"])

    in_maps, with_bias = _marshal(x, s, h, We, Ue, ve, Wk, Wr, b)
    nc = build_nc(T, with_bias=with_bias)
    res = run_bass_kernel_spmd(nc, in_maps, core_ids=list(range(NCORES)))
    out = np.concatenate([r["out"] for r in res.results], axis=0)
    return out.astype(np.float32)


if __name__ == "__main__":
    rng = np.random.default_rng(0)
    demo = {
        "x": rng.standard_normal((B, T, N), dtype=np.float32),
        "s": rng.standard_normal((B, M), dtype=np.float32) * 0.1,
        "h": rng.standard_normal((B, M), dtype=np.float32) * 0.1,
        "We": rng.standard_normal((2 * M, T), dtype=np.float32) / np.sqrt(2 * M),
        "Ue": rng.standard_normal((T, T), dtype=np.float32) / np.sqrt(T),
        "ve": rng.standard_normal((T, 1), dtype=np.float32) / np.sqrt(T),
        "Wk": rng.standard_normal((N, M4), dtype=np.float32) / np.sqrt(N),
        "Wr": rng.standard_normal((M, M4), dtype=np.float32) / np.sqrt(M),
        "b": np.zeros((M4,), dtype=np.float32),
    }
    out = kernel(**demo)
    print(out.shape, out.dtype)
